# revision 45
# baseline (speedup 1.0000x reference)
"""Causal multi-head attention on 8 trn2 NeuronCores.

Problem: B=2, S=2048, D=2048, H=16 (HD=128), fp32 in/out.
Sharding: tensor-parallel over heads - core c owns heads {2c, 2c+1} for both
batches. Each core computes its Q/K/V projections, attention for its 4
(batch, head) pairs, and a partial output projection over its head slice.
The host sums the 8 partial outputs and adds the output bias.

Device algorithm (per core), all bf16 on the PE with f32 PSUM accumulation:
  Per batch b:
    Phase A(b): stream X^T, compute Q^T/K^T (head-dim on partitions) and V
       (tokens on partitions). All three stay SBUF-resident (bf16 halves the
       footprint vs f32, so no DRAM spill of K^T is needed).
    Phase B(b): per 512-token q-block, stream k-chunks of 128 in PAIRS:
       one scores matmul per chunk into a 2-bank PSUM pair tile, one ACT exp
       per pair (halves the ~350ns/instr ACT overhead), a triangular 0/1
       mask on the 128-wide diagonal boundary only, and causally-dead
       columns are never computed (per-chunk column offsets). The softmax
       denominator is a running elementwise sum of E tiles (bf16 pair-adds
       in the DVE fast path + f32 master accumulation split across DVE and
       Pool) finished by a single ones-matmul per (head, q-block) - this
       removes the per-chunk ones-matmuls that used to cost ~11% of all PE
       cycles. ctx matmuls interleave with the scores stream (lag one pair)
       so the PE never waits on ACT; each q-block's output-projection
       matmuls are deferred one block and spliced in as PE filler. Their
       PSUM results are staged to SBUF by whichever of ACT/DVE/Pool has
       slack in the current window, then DMA'd to DRAM.
No max-subtraction is needed: scores are O(5) for this problem so exp
cannot overflow, and softmax is shift-invariant.
"""

import os

import numpy as np
import ml_dtypes

import concourse.bacc as bacc
import concourse.tile as tile
from concourse import mybir
from concourse.bass_utils import run_bass_kernel_spmd

BF16 = ml_dtypes.bfloat16


def _install_neff_cache():
    """Cache compiled NEFFs on disk keyed by BIR content hash.

    Purely a compile-time memo: identical BIR -> identical NEFF, so repeat
    runs skip the multi-minute neuronxcc compile. No effect on execution.
    """
    import hashlib
    import shutil

    import concourse.bass2jax as _b2j
    import concourse.bass_utils as _bu

    if getattr(_bu, "_neff_cache_installed", False):
        return
    cache_dir = os.environ.get("NEFF_CACHE_DIR", "/tmp/neff_cache")
    orig = _bu.compile_bir_kernel

    def cached(bir_json, tmpdir, neff_name="file.neff"):
        try:
            os.makedirs(cache_dir, exist_ok=True)
            key = hashlib.sha256(bir_json).hexdigest()[:24]
            cpath = os.path.join(cache_dir, key + ".neff")
            dst = os.path.join(tmpdir, neff_name)
            if os.path.exists(cpath):
                shutil.copy(cpath, dst)
                return dst
            out = orig(bir_json, tmpdir, neff_name)
            shutil.copy(out, cpath)
            return out
        except OSError:
            return orig(bir_json, tmpdir, neff_name)

    _bu.compile_bir_kernel = cached
    _b2j.compile_bir_kernel = cached
    _bu._neff_cache_installed = True


_install_neff_cache()

B, S, D, H = 2, 2048, 2048, 16
HD = D // H          # 128
NCORES = 8
HPC = H // NCORES    # heads per core = 2
T = B * S            # 4096 total token rows
KO = D // 128        # 16 contraction chunks
NQB = S // 512       # q-blocks per batch
SCALE = 1.0 / float(np.sqrt(HD))

# staging-copy engine mix per window (how outproj PSUM tiles reach SBUF):
# only ACT and DVE may read PSUM (GpSimd/Pool cannot). ACT is idle in
# phase-A windows and during small q-blocks but is the binding engine at
# qb=3, where DVE carries all copies.
_COPY_MIX = {
    "a": ["act"],
    0: ["act"],
    1: ["act", "dve"],
    2: ["act", "dve", "dve"],
    3: ["dve"],
    "tail": ["act", "dve"],
}
# how many pending outproj units each qb window may drain
_QUOTA = {0: 16, 1: 16, 2: 16, 3: 16}

_built = {}


def _build(with_bias):
    f32 = mybir.dt.float32
    bf = mybir.dt.bfloat16

    nc = bacc.Bacc(None, target_bir_lowering=False)

    xt_p = nc.declare_dram_parameter("XT", [KO, 128, T], bf, False)
    wqt_p = nc.declare_dram_parameter("WQT", [KO, 128, HPC * HD], bf, False)
    wkt_p = nc.declare_dram_parameter("WKT", [KO, 128, HPC * HD], bf, False)
    wvt_p = nc.declare_dram_parameter("WVT", [KO, 128, HPC * HD], bf, False)
    wot_p = nc.declare_dram_parameter("WOT", [128, HPC, D], bf, False)
    tri_p = nc.declare_dram_parameter("TRIMASK", [128, 128], bf, False)
    ones_p = nc.declare_dram_parameter("ONES", [128, 128], bf, False)
    if with_bias:
        bqk_p = nc.declare_dram_parameter("BQK", [128, 2, HPC], f32, False)
        bv_p = nc.declare_dram_parameter("BV", [128, HPC, HD], bf, False)
    out_p = nc.declare_dram_parameter("OUT", [B, S, D], f32, True)

    with tile.TileContext(nc) as tc:
        with (
            tc.tile_pool(name="persist", bufs=1) as persist,
            tc.tile_pool(name="xs", bufs=3) as xpool,
            tc.tile_pool(name="ps", bufs=2, space="PSUM") as ps,
            tc.tile_pool(name="ep", bufs=8) as epool,
            tc.tile_pool(name="small", bufs=2) as small,
        ):
            qt_res = persist.tile([128, B, HPC, S], bf)
            kt_res = persist.tile([128, B, HPC, S], bf)
            v_res = persist.tile([128, B, HPC, S // 128, HD], bf)

            wq = persist.tile([128, KO, HPC * HD], bf)
            wk = persist.tile([128, KO, HPC * HD], bf)
            wv = persist.tile([128, KO, HPC * HD], bf)
            wot = persist.tile([128, HPC, D], bf)
            trimask = persist.tile([128, 128], bf)
            ones = persist.tile([128, 128], bf)

            # DMA routing: XT streams on the Scalar engine's queue and the
            # bulk weights on GpSimd's, so neither sits behind the other (or
            # behind phase-B output writes, which use Sync's queue). The head
            # block's XT pieces alternate Scalar/Sync, interleaved with the
            # wq pieces they are consumed with, so the first Q chains start
            # within a couple of microseconds.
            for ko in range(KO):
                ksl = slice(ko, ko + 1)
                nc.gpsimd.dma_start(wk[:, ksl], wkt_p[ksl].rearrange("k p m -> p k m"))
                nc.gpsimd.dma_start(wv[:, ksl], wvt_p[ksl].rearrange("k p m -> p k m"))
            nc.gpsimd.dma_start(trimask, tri_p[:])
            nc.gpsimd.dma_start(ones, ones_p[:])
            nc.gpsimd.dma_start(wot, wot_p[:])
            if with_bias:
                bqk = persist.tile([128, 2, HPC], f32)
                bvt = persist.tile([128, HPC, HD], bf)
                nc.gpsimd.dma_start(bqk, bqk_p[:])
                nc.gpsimd.dma_start(bvt, bv_p[:])

            def load_xt(tb, eng=None):
                tglob = tb * 512
                eng = eng or nc.scalar
                xt = xpool.tile([128, KO, 512], bf, tag="xt", name="xt")
                for g in range(4):
                    eng.dma_start(
                        xt[:, g * 4 : (g + 1) * 4],
                        xt_p[g * 4 : (g + 1) * 4, :, tglob : tglob + 512]
                        .rearrange("k p t -> p k t"),
                    )
                return xt

            def load_xt_head():
                # head block: 8 small pieces alternating Scalar/Sync queues,
                # with the wq pieces interleaved on Sync ahead of the XT
                # pieces consumed alongside them
                xt = xpool.tile([128, KO, 512], bf, tag="xt", name="xt")
                nc.sync.dma_start(
                    wq[:, 0:4], wqt_p[0:4].rearrange("k p m -> p k m")
                )
                for g in range(8):
                    if g % 2 == 1 and g < 7:
                        wg = g // 2 + 1
                        nc.sync.dma_start(
                            wq[:, 4 * wg : 4 * wg + 4],
                            wqt_p[4 * wg : 4 * wg + 4].rearrange("k p m -> p k m"),
                        )
                    eng = nc.scalar if g % 2 == 0 else nc.sync
                    eng.dma_start(
                        xt[:, g * 2 : (g + 1) * 2],
                        xt_p[g * 2 : (g + 1) * 2, :, 0:512]
                        .rearrange("k p t -> p k t"),
                    )
                return xt

            # ---- pending output-projection units (PE filler work) ----
            # each unit: one [128 tok, 512 outdim] psum tile = 2 matmuls,
            # a staging copy on the window's least-loaded engine, and a DMA
            pending = []
            copy_state = {"mix": ["dve"], "i": 0, "alt_tag": False, "split": False}

            def emit_unit():
                b, qb, ctxs, qc, oc = pending.pop(0)
                # in phase-A/tail windows the attention ctx accumulators are
                # idle, so alternate units into their PSUM banks for a
                # 4-deep rotation (halves copy-latency stalls)
                tag = "c" if (copy_state["alt_tag"] and copy_state["i"] % 2) else "o"
                pso = ps.tile([128, 512], f32, tag=tag, name="pso")
                for h in range(HPC):
                    nc.tensor.matmul(
                        pso,
                        lhsT=ctxs[h][:, qc * 128 : (qc + 1) * 128],
                        rhs=wot[:, h, oc * 512 : (oc + 1) * 512],
                        start=(h == 0),
                        stop=(h == HPC - 1),
                    )
                ob = small.tile([128, 512], f32, tag="ob", bufs=4, name="ob")
                eng = copy_state["mix"][copy_state["i"] % len(copy_state["mix"])]
                copy_state["i"] += 1
                if copy_state["split"]:
                    # latency-critical tail: halve the staging-copy latency
                    # by splitting it across ACT and DVE
                    nc.scalar.copy(ob[:, :256], pso[:, :256])
                    nc.vector.tensor_copy(ob[:, 256:], pso[:, 256:])
                elif eng == "act":
                    nc.scalar.copy(ob, pso)
                else:
                    nc.vector.tensor_copy(ob, pso)
                r0 = qb * 512 + qc * 128
                nc.sync.dma_start(out_p[b, r0 : r0 + 128, oc * 512 : (oc + 1) * 512], ob)

            def emit_units(n):
                for _ in range(min(n, len(pending))):
                    emit_unit()

            def set_mix(key):
                copy_state["mix"] = _COPY_MIX[key]
                copy_state["i"] = 0
                copy_state["alt_tag"] = key in ("a", "tail")
                copy_state["split"] = key == "tail"

            # ---------------- Phase A for one batch ----------------
            def phase_a(b, xts, fillers_per_tb, head=False):
                set_mix("a")
                for tbl in range(4):
                    s0 = tbl * 512
                    tb = b * 4 + tbl
                    xt = xts.pop(tb) if tb in xts else load_xt(tb)
                    if head and tbl == 0:
                        # first block: interleave the Q, K and V chains at
                        # single-ko granularity, paced to the DMA arrival
                        # order (K and V lag their slower GpSimd-queue
                        # weights). V accumulates in the attention ctx
                        # banks, which are idle during the head block.
                        pq = ps.tile([128, 2, 512], f32, tag="s", name="pq")
                        pk = ps.tile([128, 2, 512], f32, tag="s", name="pk")
                        cv = [
                            ps.tile([128, HPC, HD], f32, tag=("c" if i < 2 else "o"),
                                    name="cv")
                            for i in range(4)
                        ]
                        for pos in range(KO + 4):
                            for which, ko in (("q", pos), ("k", pos - 2),
                                              ("v", pos - 4)):
                                if not (0 <= ko < KO):
                                    continue
                                if which == "v":
                                    for j in range(4):
                                        nc.tensor.matmul(
                                            cv[j].rearrange("p h d -> p (h d)"),
                                            lhsT=xt[:, ko, j * 128 : (j + 1) * 128],
                                            rhs=wv[:, ko],
                                            start=(ko == 0),
                                            stop=(ko == KO - 1),
                                        )
                                    continue
                                wt, pp = (wq, pq) if which == "q" else (wk, pk)
                                for h in range(HPC):
                                    nc.tensor.matmul(
                                        pp[:, h],
                                        lhsT=wt[:, ko, h * HD : (h + 1) * HD],
                                        rhs=xt[:, ko],
                                        start=(ko == 0),
                                        stop=(ko == KO - 1),
                                    )
                        nc.vector.tensor_copy(qt_res[:, b, :, s0 : s0 + 512], pq)
                        nc.vector.tensor_copy(kt_res[:, b, :, s0 : s0 + 512], pk)
                        for j in range(4):
                            nc.vector.tensor_copy(v_res[:, b, :, j, :], cv[j])
                        if with_bias:
                            for qk, dst in ((0, qt_res), (1, kt_res)):
                                for h in range(HPC):
                                    nc.vector.tensor_scalar_add(
                                        dst[:, b, h, s0 : s0 + 512],
                                        dst[:, b, h, s0 : s0 + 512],
                                        bqk[:, qk, h : h + 1],
                                    )
                        if with_bias:
                            for sc in range(4):
                                nc.vector.tensor_add(
                                    v_res[:, b, :, sc, :],
                                    v_res[:, b, :, sc, :],
                                    bvt,
                                )
                        continue
                    if True:
                        for qk, (wt, dst) in enumerate(((wq, qt_res), (wk, kt_res))):
                            emit_units(fillers_per_tb // 2)
                            pp = ps.tile([128, 2, 512], f32, tag="s", name="pqk")
                            for h in range(HPC):
                                for ko in range(KO):
                                    nc.tensor.matmul(
                                        pp[:, h],
                                        lhsT=wt[:, ko, h * HD : (h + 1) * HD],
                                        rhs=xt[:, ko],
                                        start=(ko == 0),
                                        stop=(ko == KO - 1),
                                    )
                            nc.vector.tensor_copy(dst[:, b, :, s0 : s0 + 512], pp)
                            if with_bias:
                                for h in range(HPC):
                                    nc.vector.tensor_scalar_add(
                                        dst[:, b, h, s0 : s0 + 512],
                                        dst[:, b, h, s0 : s0 + 512],
                                        bqk[:, qk, h : h + 1],
                                    )
                    # V: tokens on partitions; 4 chains of 16 in one pair tile
                    pv = ps.tile([128, 2, 2, HPC, HD], f32, tag="s", name="pv")
                    for j in range(4):
                        reg = pv[:, j // 2, j % 2]
                        for ko in range(KO):
                            nc.tensor.matmul(
                                reg.rearrange("p h d -> p (h d)"),
                                lhsT=xt[:, ko, j * 128 : (j + 1) * 128],
                                rhs=wv[:, ko],
                                start=(ko == 0),
                                stop=(ko == KO - 1),
                            )
                    nc.vector.tensor_copy(
                        v_res[:, b, :, 4 * tbl : 4 * tbl + 4, :]
                        .rearrange("p h (i u) d -> p i u h d", i=2),
                        pv,
                    )
                    if with_bias:
                        for sc in range(4):
                            nc.vector.tensor_add(
                                v_res[:, b, :, 4 * tbl + sc, :],
                                v_res[:, b, :, 4 * tbl + sc, :],
                                bvt,
                            )

            # ---------------- Phase B for one batch ----------------
            def phase_b(b):
                prev = {"blk": None}

                def finish_block(blk):
                    # deferred block finish: exact f32 denominator reduction
                    # on the PE, fast reciprocal, normalize, then queue the
                    # block's 16 output-projection units. The two heads'
                    # matmul chains are interleaved with their last partial
                    # sums (the most recently produced) at the very end, so
                    # the PE never head-of-line blocks on a pending pairsum.
                    b_, qb_, subs_, cC_ = blk
                    pds = [ps.tile([128, 512], f32, tag="o", name="pd")
                           for _ in range(HPC)]
                    n = len(subs_[0])
                    for h in range(HPC):
                        for k, (pr, off) in enumerate(subs_[h][: n - 1]):
                            nc.tensor.matmul(
                                pds[h][:, off:], lhsT=ones, rhs=pr[:, off:],
                                start=(k == 0), stop=False,
                            )
                    for h in range(HPC):
                        pr, off = subs_[h][n - 1]
                        nc.tensor.matmul(
                            pds[h][:, off:], lhsT=ones, rhs=pr[:, off:],
                            start=(n == 1), stop=True,
                        )
                    ctxs = []
                    for h in range(HPC):
                        rec = small.tile([128, 512], f32, tag="rec", name="rec")
                        nc.vector.reciprocal_approx_fast(rec, pds[h])
                        csb = small.tile([128, 512], bf, tag="csb", bufs=4, name="csb")
                        nc.vector.tensor_mul(csb, cC_[h], rec)
                        ctxs.append(csb)
                    for qc in range(4):
                        for oc in range(D // 512):
                            pending.append((b_, qb_, ctxs, qc, oc))

                for qb in range(NQB):
                    set_mix(qb if qb in _COPY_MIX else 1)
                    nk = 4 * (qb + 1)
                    npairs = nk // 2
                    cC = [ps.tile([128, 512], f32, tag="c", name="cC") for _ in range(HPC)]
                    subs = [[] for _ in range(HPC)]  # (bf16 partial-sum, off)
                    es = [[None] * npairs for _ in range(HPC)]
                    offs = [0 if t < 4 * qb else 128 * (t - 4 * qb) for t in range(nk)]

                    def ctx_pair(p):
                        for h in range(HPC):
                            for j in range(2):
                                t = 2 * p + j
                                o = offs[t]
                                nc.tensor.matmul(
                                    cC[h][:, o:],
                                    lhsT=v_res[:, b, h, t, :],
                                    rhs=es[h][p][:, j, o:],
                                    start=(t == 0),
                                    stop=(t == nk - 1),
                                )

                    # filler schedule: the previous block's outproj units are
                    # created at its deferred finish (during step 1 below),
                    # then spread over the remaining pair steps; for the last
                    # block of a batch, 3 are reserved for the block tail to
                    # cover the last-pairsum -> denominator latency
                    last = qb == NQB - 1
                    fill = [0] * npairs
                    rem = 16 if prev["blk"] else min(len(pending), _QUOTA[qb])
                    tail_fill = min(6, rem) if (last and npairs > 2) else 0
                    hold_back = 4 if (last and rem >= 10) else 0
                    lo = 2 if prev["blk"] else 1
                    for i in range(rem - tail_fill - hold_back):
                        fill[lo + i % max(1, npairs - lo)] += 1

                    for p in range(npairs):
                        if p == 1 and prev["blk"]:
                            finish_block(prev["blk"])
                            prev["blk"] = None
                        emit_units(fill[p])
                        ou = offs[2 * p]
                        for h in range(HPC):
                            pss = ps.tile([128, 2, 512], f32, tag="s", name="pss")
                            for j in range(2):
                                # both chunks computed at the pair's union
                                # width so the single exp below reads only
                                # written PSUM (j=1's extra 128 columns are
                                # causally dead and never consumed)
                                t = 2 * p + j
                                nc.tensor.matmul(
                                    pss[:, j, ou:],
                                    lhsT=kt_res[:, b, h, t * 128 : (t + 1) * 128],
                                    rhs=qt_res[:, b, h, qb * 512 + ou : (qb + 1) * 512],
                                    start=True,
                                    stop=True,
                                )
                            e = epool.tile([128, 2, 512], bf, tag="e", name="e")
                            nc.scalar.activation(
                                e[:, :, ou:], pss[:, :, ou:],
                                mybir.ActivationFunctionType.Exp,
                                scale=SCALE,
                            )
                            es[h][p] = e
                            for j in range(2):
                                t = 2 * p + j
                                if t >= 4 * qb:
                                    o = offs[t]
                                    nc.vector.tensor_mul(
                                        e[:, j, o : o + 128], e[:, j, o : o + 128],
                                        trimask,
                                    )
                            # softmax denominator partials: one bf16 pair-sum
                            # per pair on the DVE fast path; the cross-pair
                            # reduction happens exactly in f32 via chained
                            # ones-matmuls into PSUM at the block tail
                            pr = small.tile([128, 512], bf, tag="pr", bufs=16, name="pr")
                            if 2 * p + 1 < 4 * qb:
                                nc.vector.tensor_add(pr, e[:, 0], e[:, 1])
                                subs[h].append((pr, 0))
                            else:
                                a, bo_ = offs[2 * p], offs[2 * p + 1]
                                nc.vector.tensor_copy(pr[:, a:], e[:, 0, a:])
                                nc.vector.tensor_add(
                                    pr[:, bo_:], pr[:, bo_:], e[:, 1, bo_:]
                                )
                                subs[h].append((pr, a))
                        if p >= 2:
                            ctx_pair(p - 2)
                    ctx_pair(npairs - 2)
                    ctx_pair(npairs - 1)
                    emit_units(tail_fill)

                    prev["blk"] = (b, qb, subs, cC)
                    if last:
                        finish_block(prev["blk"])
                        prev["blk"] = None
                        # held-back units (from the previous block) cover the
                        # reciprocal/normalize chain before the final units
                        emit_units(4)

            # tb1 rides the Sync queue (idle after wq) so it lands while
            # tb0's pieces are still streaming on Scalar
            xts = {0: load_xt_head(), 1: load_xt(1, eng=nc.sync)}
            phase_a(0, xts, 0, head=True)
            xts[4] = load_xt(4)   # prefetch b1's first block behind B(b0)
            phase_b(0)
            phase_a(1, xts, 6)
            phase_b(1)
            set_mix("tail")
            emit_units(len(pending))

    nc.finalize()
    return nc


def _get_nc(with_bias=False):
    if with_bias not in _built:
        _built[with_bias] = _build(with_bias)
    return _built[with_bias]


def kernel(hidden_states, attention_mask, Wq, bq, Wk, bk, Wv, bv, Wo, bo):
    hidden_states = np.asarray(hidden_states, dtype=np.float32)
    Wq, Wk, Wv, Wo = (np.asarray(w, dtype=np.float32) for w in (Wq, Wk, Wv, Wo))
    bq, bk, bv, bo = (np.asarray(v, dtype=np.float32) for v in (bq, bk, bv, bo))

    with_bias = bool(np.any(bq) or np.any(bk) or np.any(bv))

    x = hidden_states.reshape(T, D)
    # [KO, 128, T]: XT[ko, p, t] = x[t, 128*ko + p]
    xt = np.ascontiguousarray(x.T).reshape(KO, 128, T).astype(BF16)

    tri = (np.arange(128)[:, None] <= np.arange(128)[None, :]).astype(BF16)
    ones = np.ones((128, 128), dtype=BF16)

    in_maps = []
    for c in range(NCORES):
        rows = slice(c * HPC * HD, (c + 1) * HPC * HD)
        wqt = np.ascontiguousarray(Wq[rows, :].T).reshape(KO, 128, HPC * HD).astype(BF16)
        wkt = np.ascontiguousarray(Wk[rows, :].T).reshape(KO, 128, HPC * HD).astype(BF16)
        wvt = np.ascontiguousarray(Wv[rows, :].T).reshape(KO, 128, HPC * HD).astype(BF16)
        # WOT[p, h, n] = Wo[n, c*256 + h*128 + p]
        wot = np.ascontiguousarray(
            Wo[:, rows].T.reshape(HPC, 128, D).transpose(1, 0, 2)
        ).astype(BF16)
        m = {
            "XT": xt,
            "WQT": wqt,
            "WKT": wkt,
            "WVT": wvt,
            "WOT": wot,
            "TRIMASK": tri,
            "ONES": ones,
        }
        if with_bias:
            m["BQK"] = np.ascontiguousarray(
                np.stack(
                    [bq[rows].reshape(HPC, HD).T, bk[rows].reshape(HPC, HD).T],
                    axis=1,
                )
            ).astype(np.float32)
            m["BV"] = np.ascontiguousarray(
                np.broadcast_to(bv[rows].reshape(1, HPC, HD), (128, HPC, HD))
            ).astype(BF16)
        in_maps.append(m)

    res = run_bass_kernel_spmd(_get_nc(with_bias), in_maps, list(range(NCORES)))
    out = res.results[0]["OUT"].copy()
    for c in range(1, NCORES):
        out += res.results[c]["OUT"]
    out += bo
    return out


# revision 48
# speedup vs baseline: 1.0086x; 1.0086x over previous
"""Causal multi-head attention on 8 trn2 NeuronCores.

Problem: B=2, S=2048, D=2048, H=16 (HD=128), fp32 in/out.
Sharding: tensor-parallel over heads - core c owns heads {2c, 2c+1} for both
batches. Each core computes its Q/K/V projections, attention for its 4
(batch, head) pairs, and a partial output projection over its head slice.
The host sums the 8 partial outputs and adds the output bias.

Device algorithm (per core), all bf16 on the PE with f32 PSUM accumulation:
  Per batch b:
    Phase A(b): stream X^T, compute Q^T/K^T (head-dim on partitions) and V
       (tokens on partitions). All three stay SBUF-resident (bf16 halves the
       footprint vs f32, so no DRAM spill of K^T is needed).
    Phase B(b): per 512-token q-block, stream k-chunks of 128 in PAIRS:
       one scores matmul per chunk into a 2-bank PSUM pair tile, one ACT exp
       per pair (halves the ~350ns/instr ACT overhead), a triangular 0/1
       mask on the 128-wide diagonal boundary only, and causally-dead
       columns are never computed (per-chunk column offsets). The softmax
       denominator is a running elementwise sum of E tiles (bf16 pair-adds
       in the DVE fast path + f32 master accumulation split across DVE and
       Pool) finished by a single ones-matmul per (head, q-block) - this
       removes the per-chunk ones-matmuls that used to cost ~11% of all PE
       cycles. ctx matmuls interleave with the scores stream (lag one pair)
       so the PE never waits on ACT; each q-block's output-projection
       matmuls are deferred one block and spliced in as PE filler. Their
       PSUM results are staged to SBUF by whichever of ACT/DVE/Pool has
       slack in the current window, then DMA'd to DRAM.
No max-subtraction is needed: scores are O(5) for this problem so exp
cannot overflow, and softmax is shift-invariant.
"""

import os

import numpy as np
import ml_dtypes

import concourse.bacc as bacc
import concourse.tile as tile
from concourse import mybir
from concourse.bass_utils import run_bass_kernel_spmd

BF16 = ml_dtypes.bfloat16


def _install_neff_cache():
    """Cache compiled NEFFs on disk keyed by BIR content hash.

    Purely a compile-time memo: identical BIR -> identical NEFF, so repeat
    runs skip the multi-minute neuronxcc compile. No effect on execution.
    """
    import hashlib
    import shutil

    import concourse.bass2jax as _b2j
    import concourse.bass_utils as _bu

    if getattr(_bu, "_neff_cache_installed", False):
        return
    cache_dir = os.environ.get("NEFF_CACHE_DIR", "/tmp/neff_cache")
    orig = _bu.compile_bir_kernel

    def cached(bir_json, tmpdir, neff_name="file.neff"):
        try:
            os.makedirs(cache_dir, exist_ok=True)
            key = hashlib.sha256(bir_json).hexdigest()[:24]
            cpath = os.path.join(cache_dir, key + ".neff")
            dst = os.path.join(tmpdir, neff_name)
            if os.path.exists(cpath):
                shutil.copy(cpath, dst)
                return dst
            out = orig(bir_json, tmpdir, neff_name)
            shutil.copy(out, cpath)
            return out
        except OSError:
            return orig(bir_json, tmpdir, neff_name)

    _bu.compile_bir_kernel = cached
    _b2j.compile_bir_kernel = cached
    _bu._neff_cache_installed = True


_install_neff_cache()

B, S, D, H = 2, 2048, 2048, 16
HD = D // H          # 128
NCORES = 8
HPC = H // NCORES    # heads per core = 2
T = B * S            # 4096 total token rows
KO = D // 128        # 16 contraction chunks
NQB = S // 512       # q-blocks per batch
SCALE = 1.0 / float(np.sqrt(HD))

# staging-copy engine mix per window (how outproj PSUM tiles reach SBUF):
# only ACT and DVE may read PSUM (GpSimd/Pool cannot). ACT is idle in
# phase-A windows and during small q-blocks but is the binding engine at
# qb=3, where DVE carries all copies.
_COPY_MIX = {
    "a": ["act"],
    0: ["act"],
    1: ["act", "dve"],
    2: ["act", "dve", "dve"],
    3: ["dve"],
    "tail": ["act", "dve"],
}
# how many pending outproj units each qb window may drain
_QUOTA = {0: 16, 1: 16, 2: 16, 3: 16}

_built = {}


def _build(with_bias):
    f32 = mybir.dt.float32
    bf = mybir.dt.bfloat16

    nc = bacc.Bacc(None, target_bir_lowering=False)

    xt_p = nc.declare_dram_parameter("XT", [KO, 128, T], bf, False)
    wqt_p = nc.declare_dram_parameter("WQT", [KO, 128, HPC * HD], bf, False)
    wkt_p = nc.declare_dram_parameter("WKT", [KO, 128, HPC * HD], bf, False)
    wvt_p = nc.declare_dram_parameter("WVT", [KO, 128, HPC * HD], bf, False)
    wot_p = nc.declare_dram_parameter("WOT", [128, HPC, D], bf, False)
    tri_p = nc.declare_dram_parameter("TRIMASK", [128, 128], bf, False)
    ones_p = nc.declare_dram_parameter("ONES", [128, 128], bf, False)
    if with_bias:
        bqk_p = nc.declare_dram_parameter("BQK", [128, 2, HPC], f32, False)
        bv_p = nc.declare_dram_parameter("BV", [128, HPC, HD], bf, False)
    out_p = nc.declare_dram_parameter("OUT", [B, S, D], f32, True)

    with tile.TileContext(nc) as tc:
        with (
            tc.tile_pool(name="persist", bufs=1) as persist,
            tc.tile_pool(name="xs", bufs=3) as xpool,
            tc.tile_pool(name="ps", bufs=2, space="PSUM") as ps,
            tc.tile_pool(name="ep", bufs=8) as epool,
            tc.tile_pool(name="small", bufs=2) as small,
        ):
            qt_res = persist.tile([128, B, HPC, S], bf)
            kt_res = persist.tile([128, B, HPC, S], bf)
            v_res = persist.tile([128, B, HPC, S // 128, HD], bf)

            wq = persist.tile([128, KO, HPC * HD], bf)
            wk = persist.tile([128, KO, HPC * HD], bf)
            wv = persist.tile([128, KO, HPC * HD], bf)
            wot = persist.tile([128, HPC, D], bf)
            trimask = persist.tile([128, 128], bf)
            ones = persist.tile([128, 128], bf)

            # DMA routing: XT streams on the Scalar engine's queue and the
            # bulk weights on GpSimd's, so neither sits behind the other (or
            # behind phase-B output writes, which use Sync's queue). The head
            # block's XT pieces alternate Scalar/Sync, interleaved with the
            # wq pieces they are consumed with, so the first Q chains start
            # within a couple of microseconds.
            for ko in range(KO):
                ksl = slice(ko, ko + 1)
                nc.gpsimd.dma_start(wk[:, ksl], wkt_p[ksl].rearrange("k p m -> p k m"))
                nc.gpsimd.dma_start(wv[:, ksl], wvt_p[ksl].rearrange("k p m -> p k m"))
            nc.gpsimd.dma_start(trimask, tri_p[:])
            nc.gpsimd.dma_start(ones, ones_p[:])
            nc.gpsimd.dma_start(wot, wot_p[:])
            if with_bias:
                bqk = persist.tile([128, 2, HPC], f32)
                bvt = persist.tile([128, HPC, HD], bf)
                nc.gpsimd.dma_start(bqk, bqk_p[:])
                nc.gpsimd.dma_start(bvt, bv_p[:])

            def load_xt(tb, eng=None):
                tglob = tb * 512
                eng = eng or nc.scalar
                xt = xpool.tile([128, KO, 512], bf, tag="xt", name="xt")
                for g in range(4):
                    eng.dma_start(
                        xt[:, g * 4 : (g + 1) * 4],
                        xt_p[g * 4 : (g + 1) * 4, :, tglob : tglob + 512]
                        .rearrange("k p t -> p k t"),
                    )
                return xt

            def load_xt_head():
                # head block: 8 small pieces alternating Scalar/Sync queues,
                # with the wq pieces interleaved on Sync ahead of the XT
                # pieces consumed alongside them
                xt = xpool.tile([128, KO, 512], bf, tag="xt", name="xt")
                nc.sync.dma_start(
                    wq[:, 0:4], wqt_p[0:4].rearrange("k p m -> p k m")
                )
                for g in range(8):
                    if g % 2 == 1 and g < 7:
                        wg = g // 2 + 1
                        nc.sync.dma_start(
                            wq[:, 4 * wg : 4 * wg + 4],
                            wqt_p[4 * wg : 4 * wg + 4].rearrange("k p m -> p k m"),
                        )
                    eng = nc.scalar if g % 2 == 0 else nc.sync
                    eng.dma_start(
                        xt[:, g * 2 : (g + 1) * 2],
                        xt_p[g * 2 : (g + 1) * 2, :, 0:512]
                        .rearrange("k p t -> p k t"),
                    )
                return xt

            # ---- pending output-projection units (PE filler work) ----
            # each unit: one [128 tok, 512 outdim] psum tile = 2 matmuls,
            # a staging copy on the window's least-loaded engine, and a DMA
            pending = []
            copy_state = {"mix": ["dve"], "i": 0, "alt_tag": False, "split": False}

            def emit_unit():
                b, qb, ctxs, qc, oc = pending.pop(0)
                # in phase-A/tail windows the attention ctx accumulators are
                # idle, so alternate units into their PSUM banks for a
                # 4-deep rotation (halves copy-latency stalls)
                tag = "c" if (copy_state["alt_tag"] and copy_state["i"] % 2) else "o"
                pso = ps.tile([128, 512], f32, tag=tag, name="pso")
                for h in range(HPC):
                    nc.tensor.matmul(
                        pso,
                        lhsT=ctxs[h][:, qc * 128 : (qc + 1) * 128],
                        rhs=wot[:, h, oc * 512 : (oc + 1) * 512],
                        start=(h == 0),
                        stop=(h == HPC - 1),
                    )
                ob = small.tile([128, 512], f32, tag="ob", bufs=4, name="ob")
                eng = copy_state["mix"][copy_state["i"] % len(copy_state["mix"])]
                copy_state["i"] += 1
                if copy_state["split"]:
                    # latency-critical tail: halve the staging-copy latency
                    # by splitting it across ACT and DVE
                    nc.scalar.copy(ob[:, :256], pso[:, :256])
                    nc.vector.tensor_copy(ob[:, 256:], pso[:, 256:])
                elif eng == "act":
                    nc.scalar.copy(ob, pso)
                else:
                    nc.vector.tensor_copy(ob, pso)
                r0 = qb * 512 + qc * 128
                nc.sync.dma_start(out_p[b, r0 : r0 + 128, oc * 512 : (oc + 1) * 512], ob)

            def emit_units(n):
                for _ in range(min(n, len(pending))):
                    emit_unit()

            def set_mix(key):
                copy_state["mix"] = _COPY_MIX[key]
                copy_state["i"] = 0
                copy_state["alt_tag"] = key in ("a", "tail")
                copy_state["split"] = key == "tail"

            # ---------------- Phase A for one batch ----------------
            def phase_a(b, xts, fillers_per_tb, head=False):
                set_mix("a")
                for tbl in range(4):
                    s0 = tbl * 512
                    tb = b * 4 + tbl
                    xt = xts.pop(tb) if tb in xts else load_xt(tb)
                    if head and tbl == 0:
                        # first block: interleave the Q, K and V chains at
                        # single-ko granularity, paced to the DMA arrival
                        # order (K and V lag their slower GpSimd-queue
                        # weights). V accumulates in the attention ctx
                        # banks, which are idle during the head block.
                        pq = ps.tile([128, 2, 512], f32, tag="s", name="pq")
                        pk = ps.tile([128, 2, 512], f32, tag="s", name="pk")
                        cv = [
                            ps.tile([128, HPC, HD], f32, tag=("c" if i < 2 else "o"),
                                    name="cv")
                            for i in range(4)
                        ]
                        for pos in range(KO + 4):
                            for which, ko in (("q", pos), ("k", pos - 2),
                                              ("v", pos - 4)):
                                if not (0 <= ko < KO):
                                    continue
                                if which == "v":
                                    for j in range(4):
                                        nc.tensor.matmul(
                                            cv[j].rearrange("p h d -> p (h d)"),
                                            lhsT=xt[:, ko, j * 128 : (j + 1) * 128],
                                            rhs=wv[:, ko],
                                            start=(ko == 0),
                                            stop=(ko == KO - 1),
                                        )
                                    continue
                                wt, pp = (wq, pq) if which == "q" else (wk, pk)
                                for h in range(HPC):
                                    nc.tensor.matmul(
                                        pp[:, h],
                                        lhsT=wt[:, ko, h * HD : (h + 1) * HD],
                                        rhs=xt[:, ko],
                                        start=(ko == 0),
                                        stop=(ko == KO - 1),
                                    )
                        nc.vector.tensor_copy(qt_res[:, b, :, s0 : s0 + 512], pq)
                        nc.vector.tensor_copy(kt_res[:, b, :, s0 : s0 + 512], pk)
                        for j in range(4):
                            nc.vector.tensor_copy(v_res[:, b, :, j, :], cv[j])
                        if with_bias:
                            for qk, dst in ((0, qt_res), (1, kt_res)):
                                for h in range(HPC):
                                    nc.vector.tensor_scalar_add(
                                        dst[:, b, h, s0 : s0 + 512],
                                        dst[:, b, h, s0 : s0 + 512],
                                        bqk[:, qk, h : h + 1],
                                    )
                        if with_bias:
                            for sc in range(4):
                                nc.vector.tensor_add(
                                    v_res[:, b, :, sc, :],
                                    v_res[:, b, :, sc, :],
                                    bvt,
                                )
                        continue
                    def qk_chain(wt, dst, qk, split_cast=False):
                        pp = ps.tile([128, 2, 512], f32, tag="s", name="pqk")
                        for h in range(HPC):
                            for ko in range(KO):
                                nc.tensor.matmul(
                                    pp[:, h],
                                    lhsT=wt[:, ko, h * HD : (h + 1) * HD],
                                    rhs=xt[:, ko],
                                    start=(ko == 0),
                                    stop=(ko == KO - 1),
                                )
                        if split_cast:
                            # halve cast latency across ACT+DVE: phase B's
                            # first scores recycle this PSUM buffer and wait
                            # on this cast at the A->B boundary
                            nc.scalar.copy(dst[:, b, 0, s0 : s0 + 512], pp[:, 0])
                            nc.vector.tensor_copy(dst[:, b, 1, s0 : s0 + 512], pp[:, 1])
                        else:
                            nc.vector.tensor_copy(dst[:, b, :, s0 : s0 + 512], pp)
                        if with_bias:
                            for h in range(HPC):
                                nc.vector.tensor_scalar_add(
                                    dst[:, b, h, s0 : s0 + 512],
                                    dst[:, b, h, s0 : s0 + 512],
                                    bqk[:, qk, h : h + 1],
                                )

                    def v_chains():
                        pv = ps.tile([128, 2, 2, HPC, HD], f32, tag="s", name="pv")
                        for j in range(4):
                            reg = pv[:, j // 2, j % 2]
                            for ko in range(KO):
                                nc.tensor.matmul(
                                    reg.rearrange("p h d -> p (h d)"),
                                    lhsT=xt[:, ko, j * 128 : (j + 1) * 128],
                                    rhs=wv[:, ko],
                                    start=(ko == 0),
                                    stop=(ko == KO - 1),
                                )
                        nc.vector.tensor_copy(
                            v_res[:, b, :, 4 * tbl : 4 * tbl + 4, :]
                            .rearrange("p h (i u) d -> p i u h d", i=2),
                            pv,
                        )
                        if with_bias:
                            for sc in range(4):
                                nc.vector.tensor_add(
                                    v_res[:, b, :, 4 * tbl + sc, :],
                                    v_res[:, b, :, 4 * tbl + sc, :],
                                    bvt,
                                )

                    if tbl < 3:
                        emit_units(fillers_per_tb // 2)
                        qk_chain(wq, qt_res, 0)
                        emit_units(fillers_per_tb // 2)
                        qk_chain(wk, kt_res, 1)
                        v_chains()
                    else:
                        # last block before attention: K goes last with a
                        # split (fast) cast so phase B's first scores aren't
                        # stuck behind a serial DVE cast chain
                        emit_units(fillers_per_tb // 2)
                        qk_chain(wq, qt_res, 0)
                        emit_units(fillers_per_tb // 2)
                        v_chains()
                        qk_chain(wk, kt_res, 1, split_cast=True)

            # ---------------- Phase B for one batch ----------------
            def phase_b(b):
                prev = {"blk": None}

                def finish_block(blk):
                    # deferred block finish: exact f32 denominator reduction
                    # on the PE, fast reciprocal, normalize, then queue the
                    # block's 16 output-projection units. The two heads'
                    # matmul chains are interleaved with their last partial
                    # sums (the most recently produced) at the very end, so
                    # the PE never head-of-line blocks on a pending pairsum.
                    b_, qb_, subs_, cC_ = blk
                    pds = [ps.tile([128, 512], f32, tag="o", name="pd")
                           for _ in range(HPC)]
                    n = len(subs_[0])
                    for h in range(HPC):
                        for k, (pr, off) in enumerate(subs_[h][: n - 1]):
                            nc.tensor.matmul(
                                pds[h][:, off:], lhsT=ones, rhs=pr[:, off:],
                                start=(k == 0), stop=False,
                            )
                    for h in range(HPC):
                        pr, off = subs_[h][n - 1]
                        nc.tensor.matmul(
                            pds[h][:, off:], lhsT=ones, rhs=pr[:, off:],
                            start=(n == 1), stop=True,
                        )
                    ctxs = []
                    for h in range(HPC):
                        rec = small.tile([128, 512], f32, tag="rec", name="rec")
                        nc.vector.reciprocal_approx_fast(rec, pds[h])
                        csb = small.tile([128, 512], bf, tag="csb", bufs=4, name="csb")
                        nc.vector.tensor_mul(csb, cC_[h], rec)
                        ctxs.append(csb)
                    for qc in range(4):
                        for oc in range(D // 512):
                            pending.append((b_, qb_, ctxs, qc, oc))

                for qb in range(NQB):
                    set_mix(qb if qb in _COPY_MIX else 1)
                    nk = 4 * (qb + 1)
                    npairs = nk // 2
                    cC = [ps.tile([128, 512], f32, tag="c", name="cC") for _ in range(HPC)]
                    subs = [[] for _ in range(HPC)]  # (bf16 partial-sum, off)
                    es = [[None] * npairs for _ in range(HPC)]
                    offs = [0 if t < 4 * qb else 128 * (t - 4 * qb) for t in range(nk)]

                    def ctx_pair(p):
                        for h in range(HPC):
                            for j in range(2):
                                t = 2 * p + j
                                o = offs[t]
                                nc.tensor.matmul(
                                    cC[h][:, o:],
                                    lhsT=v_res[:, b, h, t, :],
                                    rhs=es[h][p][:, j, o:],
                                    start=(t == 0),
                                    stop=(t == nk - 1),
                                )

                    # filler schedule: the previous block's outproj units are
                    # created at its deferred finish (during step 1 below),
                    # then spread over the remaining pair steps; for the last
                    # block of a batch, 3 are reserved for the block tail to
                    # cover the last-pairsum -> denominator latency
                    last = qb == NQB - 1
                    fill = [0] * npairs
                    rem = 16 if prev["blk"] else min(len(pending), _QUOTA[qb])
                    tail_fill = min(6, rem) if (last and npairs > 2) else 0
                    hold_back = 4 if (last and rem >= 10) else 0
                    lo = 2 if prev["blk"] else 1
                    for i in range(rem - tail_fill - hold_back):
                        fill[lo + i % max(1, npairs - lo)] += 1

                    for p in range(npairs):
                        if p == 1 and prev["blk"]:
                            finish_block(prev["blk"])
                            prev["blk"] = None
                        emit_units(fill[p])
                        ou = offs[2 * p]
                        for h in range(HPC):
                            pss = ps.tile([128, 2, 512], f32, tag="s", name="pss")
                            for j in range(2):
                                # both chunks computed at the pair's union
                                # width so the single exp below reads only
                                # written PSUM (j=1's extra 128 columns are
                                # causally dead and never consumed)
                                t = 2 * p + j
                                nc.tensor.matmul(
                                    pss[:, j, ou:],
                                    lhsT=kt_res[:, b, h, t * 128 : (t + 1) * 128],
                                    rhs=qt_res[:, b, h, qb * 512 + ou : (qb + 1) * 512],
                                    start=True,
                                    stop=True,
                                )
                            e = epool.tile([128, 2, 512], bf, tag="e", name="e")
                            nc.scalar.activation(
                                e[:, :, ou:], pss[:, :, ou:],
                                mybir.ActivationFunctionType.Exp,
                                scale=SCALE,
                            )
                            es[h][p] = e
                            for j in range(2):
                                t = 2 * p + j
                                if t >= 4 * qb:
                                    o = offs[t]
                                    nc.vector.tensor_mul(
                                        e[:, j, o : o + 128], e[:, j, o : o + 128],
                                        trimask,
                                    )
                            # softmax denominator partials: one bf16 pair-sum
                            # per pair on the DVE fast path; the cross-pair
                            # reduction happens exactly in f32 via chained
                            # ones-matmuls into PSUM at the block tail
                            pr = small.tile([128, 512], bf, tag="pr", bufs=16, name="pr")
                            if 2 * p + 1 < 4 * qb:
                                nc.vector.tensor_add(pr, e[:, 0], e[:, 1])
                                subs[h].append((pr, 0))
                            else:
                                a, bo_ = offs[2 * p], offs[2 * p + 1]
                                nc.vector.tensor_copy(pr[:, a:], e[:, 0, a:])
                                nc.vector.tensor_add(
                                    pr[:, bo_:], pr[:, bo_:], e[:, 1, bo_:]
                                )
                                subs[h].append((pr, a))
                        if p > 0:
                            ctx_pair(p - 1)
                    ctx_pair(npairs - 1)
                    emit_units(tail_fill)

                    prev["blk"] = (b, qb, subs, cC)
                    if last:
                        finish_block(prev["blk"])
                        prev["blk"] = None
                        # held-back units (from the previous block) cover the
                        # reciprocal/normalize chain before the final units
                        emit_units(4)

            xts = {0: load_xt_head()}
            phase_a(0, xts, 0, head=True)
            xts[4] = load_xt(4)   # prefetch b1's first block behind B(b0)
            phase_b(0)
            phase_a(1, xts, 6)
            phase_b(1)
            set_mix("tail")
            emit_units(len(pending))

    nc.finalize()
    return nc


def _get_nc(with_bias=False):
    if with_bias not in _built:
        _built[with_bias] = _build(with_bias)
    return _built[with_bias]


def kernel(hidden_states, attention_mask, Wq, bq, Wk, bk, Wv, bv, Wo, bo):
    hidden_states = np.asarray(hidden_states, dtype=np.float32)
    Wq, Wk, Wv, Wo = (np.asarray(w, dtype=np.float32) for w in (Wq, Wk, Wv, Wo))
    bq, bk, bv, bo = (np.asarray(v, dtype=np.float32) for v in (bq, bk, bv, bo))

    with_bias = bool(np.any(bq) or np.any(bk) or np.any(bv))

    x = hidden_states.reshape(T, D)
    # [KO, 128, T]: XT[ko, p, t] = x[t, 128*ko + p]
    xt = np.ascontiguousarray(x.T).reshape(KO, 128, T).astype(BF16)

    tri = (np.arange(128)[:, None] <= np.arange(128)[None, :]).astype(BF16)
    ones = np.ones((128, 128), dtype=BF16)

    in_maps = []
    for c in range(NCORES):
        rows = slice(c * HPC * HD, (c + 1) * HPC * HD)
        wqt = np.ascontiguousarray(Wq[rows, :].T).reshape(KO, 128, HPC * HD).astype(BF16)
        wkt = np.ascontiguousarray(Wk[rows, :].T).reshape(KO, 128, HPC * HD).astype(BF16)
        wvt = np.ascontiguousarray(Wv[rows, :].T).reshape(KO, 128, HPC * HD).astype(BF16)
        # WOT[p, h, n] = Wo[n, c*256 + h*128 + p]
        wot = np.ascontiguousarray(
            Wo[:, rows].T.reshape(HPC, 128, D).transpose(1, 0, 2)
        ).astype(BF16)
        m = {
            "XT": xt,
            "WQT": wqt,
            "WKT": wkt,
            "WVT": wvt,
            "WOT": wot,
            "TRIMASK": tri,
            "ONES": ones,
        }
        if with_bias:
            m["BQK"] = np.ascontiguousarray(
                np.stack(
                    [bq[rows].reshape(HPC, HD).T, bk[rows].reshape(HPC, HD).T],
                    axis=1,
                )
            ).astype(np.float32)
            m["BV"] = np.ascontiguousarray(
                np.broadcast_to(bv[rows].reshape(1, HPC, HD), (128, HPC, HD))
            ).astype(BF16)
        in_maps.append(m)

    res = run_bass_kernel_spmd(_get_nc(with_bias), in_maps, list(range(NCORES)))
    out = res.results[0]["OUT"].copy()
    for c in range(1, NCORES):
        out += res.results[c]["OUT"]
    out += bo
    return out
